# revision 13
# baseline (speedup 1.0000x reference)
"""AugmentedTripletLoss on 8 TRN2 NeuronCores — grouped data-parallel kernel.

v9 "dots-only" design: host sorts samples by class and pads each class
to a multiple of 512 (fixed capacity 34 chunks x 512 per core), so every
chunk is class-pure. The normalized embeddings stream in transposed
grouped layout [d_p, (chunk, k, s')] as fp8 e4m3 across both HWDGE
rings. Per chunk: two DoubleRow matmuls with the tiny chT stationary
give dots [16, 512] in PSUM (streaming-heavy PE, full HAM clock); the
nonlinear inter term is reduced on-chip, split between ACT (Relu with
accum_out) and DVE (add/max/reduce chain) so neither engine binds.
Linear terms (class sums, d_own) are host-side segment sums of the same
fp8 data, like the baseline's host-side normalization. Host combines:
S = red @ chunk-one-hot minus exact pad corrections (each pad
contributes relu(beta-1)=0.1), intra is linear for unit-norm data,
pm/deg/final in fp64. No collectives.
"""

import sys

sys.path.insert(0, "/opt/trn_rl_repo")

import numpy as np

import concourse.bass as bass
import concourse.bacc as bacc
import concourse.tile as tile
import concourse.mybir as mybir
from concourse.bass_utils import run_bass_kernel_spmd

ALPHA = 0.1
BETA = 1.1
C = 16
N = 131072
D = 512
CORES = 8
P = 128
CH = 512            # samples per class-pure chunk
CAP = 34            # chunks per core (fixed capacity, 11-sigma headroom)
NL2 = CAP * CH      # 17408 samples per core (incl. pads)
T2 = NL2 // P       # 136 tiles per core
KCH = D // P        # 4 contraction chunks of 128
GT = 8              # tiles (= 2 chunks) per DMA group
NG = T2 // GT       # 17 groups
ACT_CHUNKS = 22     # chunks reduced on ACT; the rest go to DVE
EPS = 1e-8

F32 = mybir.dt.float32
FP8 = mybir.dt.float8e4
ALU = mybir.AluOpType
ACTF = mybir.ActivationFunctionType
DR = mybir.MatmulPerfMode.DoubleRow

_CACHE = {}


def _build():
    nc = bacc.Bacc("TRN2", target_bir_lowering=False, debug=False, num_devices=CORES)

    ehatT = nc.dram_tensor("ehatT", [P, T2 * D], FP8, kind="ExternalInput")
    chTin = nc.dram_tensor("chTin", [P, KCH * C], FP8, kind="ExternalInput")
    out = nc.dram_tensor("out", [C, CAP], F32, kind="ExternalOutput")

    with tile.TileContext(nc) as tc:
        with (
            tc.tile_pool(name="pers", bufs=1) as pers,
            tc.tile_pool(name="work", bufs=6) as work,
            tc.tile_pool(name="ld", bufs=17) as ld,
            tc.tile_pool(name="psacc", bufs=1, space="PSUM") as psacc,
            tc.tile_pool(name="pstr", bufs=4, space="PSUM") as pstr,
        ):
            eT = pers.tile([P, T2 * D], FP8)    # [p, (chunk, k, s')] normalized
            chT = pers.tile([P, KCH * C], FP8)
            bq = pers.tile([P, 1], F32)
            red = pers.tile([C, CAP], F32)

            nc.gpsimd.dma_start(chT[:], chTin[:, :])
            nc.vector.memset(bq[:], float(BETA - 1.0))

            for g in range(NG):
                eng = nc.sync if g % 2 == 0 else nc.scalar
                eng.dma_start(eT[:, g * GT * D:(g + 1) * GT * D],
                              ehatT[:, g * GT * D:(g + 1) * GT * D])

            def emit_dots(j, on_act):
                dotT = pstr.tile([C, CH], F32, tag="dot")
                for kk in range(2):
                    nc.tensor.matmul(
                        dotT[:],
                        chT[:, kk * 2 * C:(kk + 1) * 2 * C]
                           .rearrange("p (two c) -> p two c", two=2),
                        eT[:, j * 2048 + kk * 1024:j * 2048 + (kk + 1) * 1024]
                           .rearrange("p (two s) -> p two s", two=2),
                        start=(kk == 0), stop=(kk == 1), perf_mode=DR)
                if on_act:
                    qsb = work.tile([C, CH], FP8)
                    nc.scalar.activation(qsb[:], dotT[:], ACTF.Relu,
                                         bias=bq[:C, :], scale=1.0,
                                         accum_out=red[:, j:j + 1])
                else:
                    qv = work.tile([C, CH], F32, tag="qv")
                    nc.vector.tensor_scalar(qv[:], dotT[:],
                                            float(BETA - 1.0), None, ALU.add)
                    nc.vector.tensor_scalar_max(qv[:], qv[:], 0.0)
                    nc.vector.tensor_reduce(red[:, j:j + 1], qv[:],
                                            mybir.AxisListType.X, ALU.add)

            # dots lag one DMA group behind so the PE never waits on the
            # group that is currently landing; alternate the nonlinear
            # reduction between ACT and DVE to balance the engines.
            nact = [0]

            def reduce_engine(j):
                use_act = (j % 3 != 2) if nact[0] < ACT_CHUNKS else False
                if use_act:
                    nact[0] += 1
                return use_act

            for g in range(1, NG):
                emit_dots(2 * (g - 1), reduce_engine(2 * (g - 1)))
                emit_dots(2 * (g - 1) + 1, reduce_engine(2 * (g - 1) + 1))
            emit_dots(2 * (NG - 1), reduce_engine(2 * (NG - 1)))
            emit_dots(2 * (NG - 1) + 1, False)
            nc.gpsimd.dma_start(out.ap()[:, :], red[:])

    nc.compile()
    return nc


def prep(embeddings: np.ndarray, labels: np.ndarray):
    import ml_dtypes

    f8 = ml_dtypes.float8_e4m3
    embf = np.asarray(embeddings, dtype=np.float32)
    e8 = embf.astype(f8)
    e8f = e8.astype(np.float32)
    nrm = np.maximum(np.sqrt((e8f * e8f).sum(1, keepdims=True)), EPS)
    ehat8 = (e8f / nrm).astype(f8)
    labi = np.asarray(labels).astype(np.int64)

    if "nc" not in _CACHE:
        _CACHE["nc"] = _build()
    nc = _CACHE["nc"]

    cnt = np.bincount(labi, minlength=C)
    _CACHE["cnt_global"] = cnt.astype(np.float64)

    # host-side centroid direction (from the fp8-rounded embeddings)
    oh = np.zeros((N, C), np.float32)
    oh[np.arange(N), labi] = 1.0
    sums_h = e8f.T @ oh
    cent = (sums_h / np.maximum(cnt, 1.0)[None, :]).T
    cn = cent / np.maximum(np.linalg.norm(cent, axis=1, keepdims=True), EPS)
    chat8 = cn.astype(f8)
    chT_np = np.ascontiguousarray(
        chat8.reshape(C, KCH, P).transpose(2, 1, 0).reshape(P, KCH * C))
    ehat8f = ehat8.astype(np.float32)
    sums_ehat = ehat8f.T @ oh  # [D, C]
    _CACHE["d_own_sum"] = np.einsum(
        "cd,dc->c", chat8.astype(np.float64), sums_ehat.astype(np.float64))

    _CACHE["sums_host"] = sums_h.T.astype(np.float64)  # [C, D] exact segment sums

    # group by class, pad each class to a multiple of CH
    order = np.argsort(labi, kind="stable")
    tot = CORES * NL2
    ge = np.zeros((tot, D), f8)
    cls_of_chunk = np.full(CORES * CAP, -1, np.int64)
    npad_of_chunk = np.zeros(CORES * CAP, np.int64)
    pos = 0
    idx = 0
    for c in range(C):
        n = int(cnt[c])
        sel = order[idx:idx + n]
        idx += n
        ge[pos:pos + n] = ehat8[sel]
        end = pos + n
        pad = (-n) % CH
        for j in range(pos // CH, (end + pad) // CH):
            cls_of_chunk[j] = c
        if pad:
            npad_of_chunk[(end + pad) // CH - 1] = pad
        pos = end + pad
    assert pos <= tot, (pos, tot)
    _CACHE["cls_of_chunk"] = cls_of_chunk
    _CACHE["npad_of_chunk"] = npad_of_chunk

    in_maps = []
    for i in range(CORES):
        sl = slice(i * NL2, (i + 1) * NL2)
        esT = np.ascontiguousarray(
            ge[sl].reshape(CAP, CH, KCH, P).transpose(3, 0, 2, 1)
            .reshape(P, T2 * D))
        in_maps.append({"ehatT": esT, "chTin": chT_np})
    return nc, in_maps


def post(res, inputs=None):
    red = np.zeros((C, CORES * CAP), np.float64)
    cls_of_chunk = _CACHE["cls_of_chunk"]
    npad_of_chunk = _CACHE["npad_of_chunk"]
    cnt = _CACHE["cnt_global"]
    for i, r in enumerate(res.results):
        red[:, i * CAP:(i + 1) * CAP] = r["out"].astype(np.float64)
    sums = _CACHE["sums_host"]
    cent = sums / np.maximum(cnt, 1.0)[:, None]
    cn = cent / np.maximum(np.linalg.norm(cent, axis=1, keepdims=True), EPS)
    pd = 1.0 - cn @ cn.T
    upper = np.triu(np.ones((C, C), bool), 1)
    present = cnt > 0
    pm = (upper & (pd <= BETA) & present[:, None] & present[None, :]).astype(
        np.float64)
    deg = pm.sum(1) + pm.sum(0)

    ohc = np.zeros((CORES * CAP, C))
    valid = cls_of_chunk >= 0
    ohc[np.nonzero(valid)[0], cls_of_chunk[valid]] = 1.0
    npad_per_class = np.zeros(C)
    np.add.at(npad_per_class, cls_of_chunk[valid], npad_of_chunk[valid])

    S = red @ ohc - (BETA - 1.0) * npad_per_class[None, :]
    d_own_sum = _CACHE["d_own_sum"]
    t_c = (1.0 - ALPHA) * cnt - d_own_sum
    inter_sum = (pm * (S + S.T)).sum()
    intra_sum = (deg * t_c).sum()
    count = (deg * cnt).sum()
    num_pairs = pm.sum()
    loss = (intra_sum + inter_sum) / max(count, 1.0) if num_pairs > 0 else 0.0
    return np.float32(loss)


def kernel(embeddings: np.ndarray, labels: np.ndarray) -> np.ndarray:
    nc, in_maps = prep(embeddings, labels)
    res = run_bass_kernel_spmd(nc, in_maps, core_ids=list(range(CORES)))
    return post(res)
